# revision 28
# baseline (speedup 1.0000x reference)
"""Trainium2 Bass kernel for nn_GATv2Layer4View (GAT message passing + inter-view MHA).

Self-contained: kernel(**inputs) -> np.ndarray [2, 4, 10000, 128] float32.

Math (faithful to reference):
  scores[e,h] = mean_bv(s_src[bv, src[e], h] + s_dst[bv, dst[e], h])   (node-separable)
  w = softmax(scores, axis=0) = ea[src]*eb[dst]/Z;  Z computed HOST-side from
  ea/eb (free) and folded into the in_proj columns.

Launch 1 (node-sharded, 1280 nodes/core), 3 phases to avoid ACT table thrash:
  A: h (d-major) for all 8 bv per tile -> leaky (ACT Lrelu) -> hl_all
  B: bv-sum tree -> att-scaled -> head-select matmul -> Exp -> ea/eb node-major
  C: h (node-major, stationary-x matmuls) * ea -> hi/lo fp8 e4m3 rows (2048 B)
Launch 2+3 fused (dst-node-range sharded): per dst tile: contiguous self-loop
  block + dma_gather of edge rows; one-hot scatter matmuls (fp8 DoubleRow, the
  one-hots are HOST-precomputed inputs); *eb -> gatT via DMA-xbar transposes
  (SBUF only, no HBM round-trip); inter-view MHA: stationary-x qkv matmuls
  (node-major out), attention middle as b-batched broadcast outer products on
  DVE (all bf16 SBUF), out_proj via xbar transpose + matmul, ACT bias epilogue.
  Output d-major bf16; host reassembles layout + dtype (host work is free).
"""

import math
import numpy as np
import ml_dtypes

import concourse.bass as bass
import concourse.bacc as bacc
import concourse.mybir as mybir
import concourse.tile as tile
from concourse.bass_utils import run_bass_kernel_spmd
from concourse.masks import make_identity

P = 128
NCORES = 8
B, V, N, FIN = 2, 4, 10000, 64
H, F = 4, 32
D = H * F                      # 128
NEG_SLOPE = 0.2

NPC = 1280                     # nodes per core
TPC = NPC // P                 # 10 tiles per core
TBLN = NCORES * NPC            # 10240 table rows (>= N, covers pad tiles)
ROW = 2048                     # bytes per table row: 1024 hi fp8 + 1024 lo fp8

FP32 = mybir.dt.float32
BF16 = mybir.dt.bfloat16
FP8 = mybir.dt.float8e4
I16 = mybir.dt.int16

BF = ml_dtypes.bfloat16
E4 = ml_dtypes.float8_e4m3

RUN_KW = {}
EXEC_TIMES = {}


# --------------------------------------------------------------------------
# host-side edge preprocessing
# --------------------------------------------------------------------------
class EdgePlan:
    pass


def prep_edges(edge_index: np.ndarray) -> EdgePlan:
    ei = np.asarray(edge_index)
    src = ei[0].astype(np.int64)
    dst = ei[1].astype(np.int64)
    order = np.argsort(dst, kind="stable")
    ss, ds = src[order], dst[order]

    n_tiles = NCORES * TPC
    bounds = np.searchsorted(ds, np.minimum(np.arange(n_tiles + 1) * P, N))
    counts = np.diff(bounds)
    cmax = int(math.ceil(counts.max() / P))
    if (cmax + 1) % 2:         # C_T = cmax+1 chunks/tile incl identity: even
        cmax += 1
    C_T = cmax + 1

    idx_all = np.full((NCORES, TPC * cmax * P), N, np.int64)   # pad -> zero row
    rel_all = np.full((NCORES, TPC * C_T * P), 200.0, np.float32)
    for c in range(NCORES):
        for t in range(TPC):
            g = c * TPC + t
            k = bounds[g + 1] - bounds[g]
            idx_all[c, t * cmax * P:t * cmax * P + k] = ss[bounds[g]:bounds[g + 1]]
            o = (t * C_T + 1) * P
            rel_all[c, o:o + k] = ds[bounds[g]:bounds[g + 1]] - g * P
            rel_all[c, t * C_T * P:t * C_T * P + P] = np.arange(P)  # identity chunk
    plan = EdgePlan()
    plan.cmax = cmax
    plan.idx16 = [np.ascontiguousarray(idx_all[c].astype(np.int16)
                                       .reshape(-1, 16).T) for c in range(NCORES)]
    # host-built one-hot scatter matrices: S[p, ci*128 + j] = (rel[ci,p] == j)
    plan.sh = []
    n_chunks = TPC * C_T
    j = np.arange(P)
    for c in range(NCORES):
        R = rel_all[c].reshape(n_chunks, P)              # [chunk, partition]
        O = (R[:, :, None] == j[None, None, :]).astype(E4)
        plan.sh.append(np.ascontiguousarray(
            O.transpose(1, 0, 2).reshape(P, n_chunks * P)))
    return plan


# --------------------------------------------------------------------------
# launch 1: node-sharded table build (hi-lo fp8 rows + node-major ea/eb)
# --------------------------------------------------------------------------
def build_l1():
    nc = bacc.Bacc("TRN2", target_bir_lowering=False, debug=False,
                   num_devices=NCORES)
    xT = nc.dram_tensor("xT", [FIN, TPC * 8 * P], BF16, kind="ExternalInput")
    wT = nc.dram_tensor("wT", [FIN, D], BF16, kind="ExternalInput")
    attc = nc.dram_tensor("attc", [P, 2], FP32, kind="ExternalInput")
    ind4 = nc.dram_tensor("ind4", [P, 4], BF16, kind="ExternalInput")
    rows_out = nc.dram_tensor("rows", [NPC, ROW], FP8, kind="ExternalOutput")
    ee_out = nc.dram_tensor("ee", [NPC, 8], FP32, kind="ExternalOutput")

    with tile.TileContext(nc) as tc:
        with tc.tile_pool(name="one", bufs=1) as one, \
             tc.tile_pool(name="sb", bufs=3) as sb, \
             tc.tile_pool(name="pk", bufs=2) as pk, \
             tc.tile_pool(name="psA", bufs=2, space="PSUM") as psA, \
             tc.tile_pool(name="psB", bufs=2, space="PSUM") as psB, \
             tc.tile_pool(name="psS", bufs=1, space="PSUM") as psS:
            idf4 = one.tile([4, 4], FP32)
            make_identity(nc, idf4[:])
            xT_sb = one.tile([FIN, TPC * 8 * P], BF16)
            nc.sync.dma_start(xT_sb[:], xT.ap()[:])
            wT_sb = one.tile([FIN, D], BF16)
            nc.sync.dma_start(wT_sb[:], wT.ap()[:])
            att_sb = one.tile([P, 2], FP32)
            nc.sync.dma_start(att_sb[:], attc.ap()[:])
            ind_sb = one.tile([P, 4], BF16)
            nc.sync.dma_start(ind_sb[:], ind4.ap()[:])
            hl_all = one.tile([P, TPC * 8 * P], BF16)
            ee_all = one.tile([P, TPC, 8], FP32)

            # phase A: d-major h + leaky
            for t in range(TPC):
                c0 = t * 8 * P
                h_ps = psA.tile([P, 8 * P], FP32, tag="h")
                nc.tensor.matmul(h_ps[:, 0:512], wT_sb[:],
                                 xT_sb[:, c0:c0 + 512], start=True, stop=True)
                nc.tensor.matmul(h_ps[:, 512:1024], wT_sb[:],
                                 xT_sb[:, c0 + 512:c0 + 1024],
                                 start=True, stop=True)
                nc.scalar.activation(hl_all[:, c0:c0 + 8 * P], h_ps[:],
                                     mybir.ActivationFunctionType.Lrelu,
                                     alpha=NEG_SLOPE)

            # phase B: scores -> ea/eb node-major
            for t in range(TPC):
                c0 = t * 8 * P
                a1 = sb.tile([P, 512], BF16, tag="a1")
                nc.vector.tensor_add(a1[:], hl_all[:, c0:c0 + 512],
                                     hl_all[:, c0 + 512:c0 + 1024])
                a2 = sb.tile([P, 256], BF16, tag="a2")
                nc.vector.tensor_add(a2[:], a1[:, 0:256], a1[:, 256:512])
                a3 = sb.tile([P, P], FP32, tag="a3")
                nc.vector.tensor_add(a3[:], a2[:, 0:128], a2[:, 128:256])
                pp = sb.tile([P, 256], BF16, tag="pp")
                nc.vector.tensor_scalar_mul(pp[:, 0:128], a3[:], att_sb[:, 0:1])
                nc.vector.tensor_scalar_mul(pp[:, 128:256], a3[:], att_sb[:, 1:2])
                s2_ps = psS.tile([4, 256], FP32, tag="s2")
                nc.tensor.matmul(s2_ps[:], ind_sb[:], pp[:],
                                 start=True, stop=True)
                ee_row = sb.tile([4, 256], FP32, tag="eer")
                nc.scalar.activation(ee_row[:], s2_ps[:],
                                     mybir.ActivationFunctionType.Exp,
                                     scale=1.0 / 8.0)
                eT_ps = psS.tile([P, 4], FP32, tag="eT")
                nc.tensor.transpose(eT_ps[:], ee_row[:, 0:128], idf4[:])
                nc.vector.tensor_copy(ee_all[:, t, 0:4], eT_ps[:])
                eT2_ps = psS.tile([P, 4], FP32, tag="eT")
                nc.tensor.transpose(eT2_ps[:], ee_row[:, 128:256], idf4[:])
                nc.vector.tensor_copy(ee_all[:, t, 4:8], eT2_ps[:])
                nc.sync.dma_start(ee_out.ap()[t * P:(t + 1) * P, :],
                                  ee_all[:, t])

            # phase C: node-major h * ea -> hi/lo fp8 rows
            for t in range(TPC):
                c0 = t * 8 * P
                ea128 = sb.tile([P, P], BF16, tag="ea128")
                nc.vector.tensor_copy(
                    ea128[:].rearrange("p (h f) -> p h f", h=H),
                    ee_all[:, t, 0:4][:, :, None].to_broadcast([P, H, F]))
                pk32 = pk.tile([P, 8 * P], FP32, tag="pk32")
                for bv in range(8):
                    hn_ps = psB.tile([P, P], FP32, tag="hn")
                    nc.tensor.matmul(hn_ps[:],
                                     xT_sb[:, c0 + bv * P:c0 + (bv + 1) * P],
                                     wT_sb[:], start=True, stop=True)
                    nc.vector.tensor_tensor(
                        out=pk32[:, bv * P:(bv + 1) * P], in0=hn_ps[:],
                        in1=ea128[:], op=mybir.AluOpType.mult)
                packed = pk.tile([P, ROW], FP8, tag="packed")
                nc.scalar.copy(packed[:, 0:1024], pk32[:])
                nc.vector.tensor_tensor(out=packed[:, 1024:2048], in0=pk32[:],
                                        in1=packed[:, 0:1024],
                                        op=mybir.AluOpType.subtract)
                nc.sync.dma_start(rows_out.ap()[t * P:(t + 1) * P, :], packed[:])
    nc.compile()
    return nc


# --------------------------------------------------------------------------
# launch 2+3 fused: edge aggregation + inter-view MHA per dst-node range
# --------------------------------------------------------------------------
def build_l23(cmax: int, has_ipb: bool):
    C_T = cmax + 1
    n_chunks = TPC * C_T
    idx_cols = TPC * cmax * 8

    nc = bacc.Bacc("TRN2", target_bir_lowering=False, debug=False,
                   num_devices=NCORES, num_swdge_queues=2)
    tbl_in = nc.dram_tensor("table", [TBLN, ROW], FP8, kind="ExternalInput")
    tblk_in = nc.dram_tensor("tblk", [NPC, ROW], FP8, kind="ExternalInput")
    idx_in = nc.dram_tensor("idx16", [16, idx_cols], I16, kind="ExternalInput")
    sh_in = nc.dram_tensor("sh", [P, n_chunks * P], FP8, kind="ExternalInput")
    ee_in = nc.dram_tensor("ee", [NPC, 8], FP32, kind="ExternalInput")
    wiz_in = nc.dram_tensor("wiz", [P, 3 * D], BF16, kind="ExternalInput")
    wo_in = nc.dram_tensor("woT", [P, D], BF16, kind="ExternalInput")
    cb_in = nc.dram_tensor("cbias", [P, 1], FP32, kind="ExternalInput")
    ipb_in = nc.dram_tensor("ipb", [1, 3 * D], FP32, kind="ExternalInput")
    o_out = nc.dram_tensor("outT", [P, B * V * NPC], BF16, kind="ExternalOutput")

    with tile.TileContext(nc) as tc:
        with tc.tile_pool(name="one", bufs=1) as one, \
             tc.tile_pool(name="gp", bufs=3) as gp, \
             tc.tile_pool(name="sbf", bufs=2) as sbf, \
             tc.tile_pool(name="mh", bufs=3) as mh, \
             tc.tile_pool(name="pp2", bufs=2) as pp2, \
             tc.tile_pool(name="accp", bufs=2, space="PSUM") as accp, \
             tc.tile_pool(name="psQ", bufs=2, space="PSUM") as psQ:
            idx_sb = one.tile([P, idx_cols], I16)
            for r in range(8):
                nc.sync.dma_start(idx_sb[16 * r:16 * (r + 1), :], idx_in.ap()[:])
            S_all = one.tile([P, n_chunks * P], FP8)
            nc.sync.dma_start(S_all[:], sh_in.ap()[:])
            ee_sb = one.tile([P, TPC, 8], FP32)
            nc.sync.dma_start(ee_sb[:],
                              ee_in.ap().rearrange("(t p) c -> p t c", p=P))
            wiz_sb = one.tile([P, 3 * D], BF16)
            nc.sync.dma_start(wiz_sb[:], wiz_in.ap()[:])
            wo_sb = one.tile([P, D], BF16)
            nc.sync.dma_start(wo_sb[:], wo_in.ap()[:])
            cb_sb = one.tile([P, 1], FP32)
            nc.sync.dma_start(cb_sb[:], cb_in.ap()[:])
            ones_sb = one.tile([P, 1], BF16)
            nc.vector.memset(ones_sb[:], 1.0)
            if has_ipb:
                ipb_row = one.tile([1, 3 * D], FP32)
                nc.sync.dma_start(ipb_row[:], ipb_in.ap()[:])
                ipb_rb = one.tile([1, 3 * D], BF16)
                nc.vector.tensor_copy(ipb_rb[:], ipb_row[:])
                ipb_sb = one.tile([P, 3 * D], BF16)
                nc.gpsimd.partition_broadcast(ipb_sb[:], ipb_rb[:])

            gatT_sb = one.tile([P, 8 * NPC], BF16)   # [d, (bv, n)]

            acc_t = {}

            def emit_scatter(t):
                # ---- gather + scatter-accumulate --------------------------
                g = gp.tile([P, C_T, ROW], FP8, tag="g")
                nc.sync.dma_start(g[:, 0, :], tblk_in.ap()[t * P:(t + 1) * P, :])
                h1 = cmax // 2
                nc.gpsimd.dma_gather(
                    out_ap=g[:, 1:1 + h1, :],
                    in_ap=tbl_in.ap()[:],
                    idxs_ap=idx_sb[:, t * cmax * 8:(t * cmax + h1) * 8],
                    num_idxs=h1 * P,
                    num_idxs_reg=h1 * P,
                    elem_size=ROW,
                    single_packet=False,
                    queue_num=0,
                )
                nc.gpsimd.dma_gather(
                    out_ap=g[:, 1 + h1:C_T, :],
                    in_ap=tbl_in.ap()[:],
                    idxs_ap=idx_sb[:, (t * cmax + h1) * 8:(t + 1) * cmax * 8],
                    num_idxs=(cmax - h1) * P,
                    num_idxs_reg=(cmax - h1) * P,
                    elem_size=ROW,
                    single_packet=False,
                    queue_num=1,
                )
                acc = accp.tile([P, 1024], FP32, tag="acc")
                npair = C_T // 2
                for j in range(npair):
                    base = (t * C_T + 2 * j) * P
                    S2 = S_all[:, base:base + 2 * P].rearrange(
                        "p (k e) -> p k e", k=2)
                    g2 = g[:, 2 * j:2 * j + 2, :]
                    first, last = (j == 0), (j == npair - 1)
                    for half in range(2):
                        nc.tensor.matmul(
                            acc[:, half * 512:(half + 1) * 512], S2,
                            g2[:, :, half * 512:(half + 1) * 512],
                            start=first, stop=False,
                            perf_mode=mybir.MatmulPerfMode.DoubleRow,
                            skip_group_check=True)
                    for half in range(2):
                        nc.tensor.matmul(
                            acc[:, half * 512:(half + 1) * 512], S2,
                            g2[:, :, 1024 + half * 512:1024 + (half + 1) * 512],
                            start=False, stop=last,
                            perf_mode=mybir.MatmulPerfMode.DoubleRow,
                            skip_group_check=True)
                acc_t[t] = acc

            stage_q = {}

            def emit_stage2(t):
                acc = acc_t.pop(t)
                # ---- finalize: * eb -> gatT via xbar transposes -----------
                eb128 = sbf.tile([P, P], BF16, tag="eb128")
                nc.vector.tensor_tensor(
                    out=eb128[:].rearrange("p (h f) -> p h f", h=H),
                    in0=ee_sb[:, t, 4:8][:, :, None].to_broadcast([P, H, F]),
                    in1=ones_sb[:][:, :, None].to_broadcast([P, H, F]),
                    op=mybir.AluOpType.mult)
                om = sbf.tile([P, 8, P], BF16, tag="om")
                nc.vector.tensor_tensor(
                    out=om[:], in0=acc[:].rearrange("p (v d) -> p v d", v=8),
                    in1=eb128[:, None, :].to_broadcast([P, 8, P]),
                    op=mybir.AluOpType.mult)
                nc.sync.dma_start_transpose(
                    gatT_sb[:].rearrange("p (v n) -> p v n", v=8)
                    [:, :, t * P:(t + 1) * P],
                    om[:].rearrange("p v d -> p (v d)"))
                # ---- inter-view MHA (both b batched) ----------------------
                # layouts chosen so (b h) collapses to one uniform-stride dim
                q_sb = mh.tile([P, V, B, P], BF16, tag="q")        # [a][b][hf]
                k_sb = mh.tile([P, B, H, V, F], BF16, tag="kk")    # [b][h][k][f]
                v_sb = mh.tile([P, B, H, F, V], BF16, tag="vv")    # [b][h][f][k]
                for b in range(B):
                    for vh in range(2):
                        qkv_ps = psQ.tile([P, 2, 512], FP32, tag="qkv")
                        for vi in range(2):
                            v = vh * 2 + vi
                            nc.tensor.matmul(
                                qkv_ps[:, vi, 0:384],
                                gatT_sb[:, (b * V + v) * NPC + t * P:
                                        (b * V + v) * NPC + (t + 1) * P],
                                wiz_sb[:], start=True, stop=True)
                        nc.vector.tensor_tensor(
                            out=q_sb[:, vh * 2:vh * 2 + 2, b, :],
                            in0=qkv_ps[:, :, 0:128],
                            in1=ones_sb[:][:, :, None].to_broadcast(
                                [P, 2, P]),
                            op=mybir.AluOpType.mult)
                        for vi in range(2):
                            v = vh * 2 + vi
                            nc.scalar.copy(
                                k_sb[:, b, :, v, :],
                                qkv_ps[:, vi, 128:256].rearrange(
                                    "p (h f) -> p h f", h=H))
                            nc.scalar.copy(
                                v_sb[:, b, :, :, v],
                                qkv_ps[:, vi, 256:384].rearrange(
                                    "p (h f) -> p h f", h=H))
                if has_ipb:
                    nc.vector.tensor_tensor(
                        out=q_sb[:].rearrange("p a b d -> p (a b) d"),
                        in0=q_sb[:].rearrange("p a b d -> p (a b) d"),
                        in1=ipb_sb[:, None, 0:128].to_broadcast(
                            [P, V * B, P]),
                        op=mybir.AluOpType.add)
                    nc.vector.tensor_tensor(
                        out=k_sb[:].rearrange("p b h k f -> p (b h) k f"),
                        in0=k_sb[:].rearrange("p b h k f -> p (b h) k f"),
                        in1=ipb_sb[:, 128:256].rearrange(
                            "p (h f) -> p h f", h=H).rearrange(
                            "p h f -> p h 1 f").to_broadcast([P, B * H, V, F]),
                        op=mybir.AluOpType.add)
                    nc.vector.tensor_tensor(
                        out=v_sb[:].rearrange("p b h f k -> p (b h) f k"),
                        in0=v_sb[:].rearrange("p b h f k -> p (b h) f k"),
                        in1=ipb_sb[:, 256:384].rearrange(
                            "p (h f) -> p h f", h=H).rearrange(
                            "p h f -> p h f 1").to_broadcast([P, B * H, F, V]),
                        op=mybir.AluOpType.add)
                stage_q[t] = (q_sb, k_sb, v_sb)

            def emit_stage3(t):
                q_sb, k_sb, v_sb = stage_q.pop(t)
                lg = mh.tile([P, V, B, H, V], FP32, tag="lg")  # [a][b][h][k]
                for a in range(V):
                    prod = pp2.tile([P, B * H, V, F], BF16, tag="prod")
                    nc.vector.tensor_tensor(
                        out=prod[:],
                        in0=q_sb[:, a].rearrange(
                            "p b (h f) -> p (b h) f", h=H)[:, :, None, :]
                            .to_broadcast([P, B * H, V, F]),
                        in1=k_sb[:].rearrange("p b h k f -> p (b h) k f"),
                        op=mybir.AluOpType.mult)
                    nc.vector.tensor_reduce(
                        out=lg[:, a].rearrange("p b h k -> p (b h) k"),
                        in_=prod[:],
                        axis=mybir.AxisListType.X, op=mybir.AluOpType.add)
                ex = mh.tile([P, V * B * H, V], FP32, tag="ex")
                nc.scalar.activation(ex[:],
                                     lg[:].rearrange("p a b h k -> p (a b h) k"),
                                     mybir.ActivationFunctionType.Exp,
                                     scale=1.0 / math.sqrt(F))
                ssum = mh.tile([P, V * B * H], FP32, tag="ssum")
                nc.vector.tensor_reduce(
                    out=ssum[:], in_=ex[:],
                    axis=mybir.AxisListType.X, op=mybir.AluOpType.add)
                rcp = mh.tile([P, V * B * H], FP32, tag="rcp")
                nc.vector.reciprocal(rcp[:], ssum[:])
                at = mh.tile([P, V * B * H, V], BF16, tag="at")
                nc.vector.tensor_tensor(
                    out=at[:], in0=ex[:],
                    in1=rcp[:, :, None].to_broadcast([P, V * B * H, V]),
                    op=mybir.AluOpType.mult)
                obf = mh.tile([P, B, V, P], BF16, tag="obf")
                atv = at[:].rearrange("p (a bh) k -> p a bh k", a=V)
                for a in range(V):
                    pv = pp2.tile([P, B * H, F, V], BF16, tag="pv")
                    nc.vector.tensor_tensor(
                        out=pv[:],
                        in0=atv[:, a][:, :, None, :].to_broadcast(
                            [P, B * H, F, V]),
                        in1=v_sb[:].rearrange("p b h f k -> p (b h) f k"),
                        op=mybir.AluOpType.mult)
                    with nc.allow_low_precision("4-term AV sum feeds bf16 out_proj"):
                        nc.vector.tensor_reduce(
                            out=obf[:, :, a, :],
                            in_=pv[:].rearrange("p bh f k -> p (bh f) k"),
                            axis=mybir.AxisListType.X, op=mybir.AluOpType.add)
                oT_sb = mh.tile([P, B, V, P], BF16, tag="oT")   # [d][b][a][n]
                nc.scalar.dma_start_transpose(
                    oT_sb[:].rearrange("p b a n -> p (b a) n"),
                    obf[:].rearrange("p b a d -> p (b a d)"))
                fin_ps = psQ.tile([P, 2, 512], FP32, tag="qkv")
                for b in range(B):
                    nc.tensor.matmul(fin_ps[:, b], wo_sb[:],
                                     oT_sb[:, b].rearrange("p a n -> p (a n)"),
                                     start=True, stop=True)
                outb = mh.tile([P, B, V, P], BF16, tag="outb")
                for b in range(B):
                    nc.scalar.activation(
                        outb[:, b].rearrange("p v n -> p (v n)"),
                        fin_ps[:, b],
                        mybir.ActivationFunctionType.Identity,
                        bias=cb_sb[:, 0:1])
                nc.scalar.dma_start(
                    o_out.ap().rearrange("p (b v n) -> p b v n",
                                         b=B, v=V)[:, :, :, t * P:(t + 1) * P],
                    outb[:])

            for t in range(TPC):
                emit_scatter(t)
                if t >= 1:
                    emit_stage2(t - 1)
                if t >= 2:
                    emit_stage3(t - 2)
            emit_stage2(TPC - 1)
            emit_stage3(TPC - 2)
            emit_stage3(TPC - 1)
    nc.compile()
    return nc


# --------------------------------------------------------------------------
# host orchestration
# --------------------------------------------------------------------------
_cache = {}


def _get(name, builder, *args):
    if name not in _cache:
        _cache[name] = builder(*args)
    return _cache[name]


def kernel(x, W, att, in_proj_w, in_proj_b, out_proj_w, out_proj_b, bias,
           edge_index):
    x = np.asarray(x, np.float32)
    W = np.asarray(W, np.float32)
    att = np.asarray(att, np.float32)
    in_proj_w = np.asarray(in_proj_w, np.float32)
    in_proj_b = np.asarray(in_proj_b, np.float32)
    out_proj_w = np.asarray(out_proj_w, np.float32)
    out_proj_b = np.asarray(out_proj_b, np.float32)
    bias = np.asarray(bias, np.float32)
    ei = np.asarray(edge_index)

    plan_key = ei.tobytes()
    if ("plan", plan_key) not in _cache:
        _cache[("plan", plan_key)] = prep_edges(ei)
    plan = _cache[("plan", plan_key)]

    # ---- launch 1 ----
    nc1 = _get("l1", build_l1)
    xf = x.reshape(NCORES, N, FIN)
    xpad = np.zeros((NCORES, TBLN, FIN), BF)
    xpad[:, :N, :] = xf.astype(BF)
    wT = np.ascontiguousarray(W.T.astype(BF))
    attc = np.zeros((P, 2), np.float32)
    attc[:, 0] = att[0, :, :F].reshape(-1)
    attc[:, 1] = att[0, :, F:].reshape(-1)
    ind4 = np.zeros((P, 4), BF)
    for h in range(H):
        ind4[h * F:(h + 1) * F, h] = 1.0
    in1 = []
    for c in range(NCORES):
        sl = xpad[:, c * NPC:(c + 1) * NPC, :]            # [8, NPC, 64]
        xT_c = np.ascontiguousarray(
            sl.reshape(8, TPC, P, FIN).transpose(3, 1, 0, 2).reshape(FIN, -1))
        in1.append({"xT": xT_c, "wT": wT, "attc": attc, "ind4": ind4})
    r1 = run_bass_kernel_spmd(nc1, in1, core_ids=list(range(NCORES)), **RUN_KW)
    EXEC_TIMES["launch1"] = r1.exec_time_ns

    # ---- host: Z + folded weights ----
    table = np.concatenate([r1.results[c]["rows"] for c in range(NCORES)])
    ee = np.concatenate([r1.results[c]["ee"] for c in range(NCORES)])  # [TBLN,8]
    ea = ee[:N, 0:4].astype(np.float64)
    eb = ee[:N, 4:8].astype(np.float64)
    src = np.concatenate([ei[0].astype(np.int64), np.arange(N)])
    dst = np.concatenate([ei[1].astype(np.int64), np.arange(N)])
    Z = (ea[src] * eb[dst]).sum(axis=0)                   # [H]
    rz = (1.0 / Z).astype(np.float32)
    rzvec = rz[np.arange(D) // F]                         # [128]
    wiz = np.ascontiguousarray((in_proj_w.T * rzvec[:, None]).astype(BF))
    woT = np.ascontiguousarray(out_proj_w.T.astype(BF))
    cbias = np.ascontiguousarray((out_proj_b + bias).reshape(P, 1))
    ipb = np.ascontiguousarray(in_proj_b.reshape(1, 3 * D))
    has_ipb = bool(np.any(in_proj_b))

    # ---- launch 2+3 fused ----
    nc2 = _get(("l23", plan.cmax, has_ipb), build_l23, plan.cmax, has_ipb)
    in2 = []
    for c in range(NCORES):
        in2.append({"table": table, "tblk": table[c * NPC:(c + 1) * NPC],
                    "idx16": plan.idx16[c], "sh": plan.sh[c],
                    "ee": ee[c * NPC:(c + 1) * NPC], "wiz": wiz, "woT": woT,
                    "cbias": cbias, "ipb": ipb})
    r2 = run_bass_kernel_spmd(nc2, in2, core_ids=list(range(NCORES)), **RUN_KW)
    EXEC_TIMES["launch23"] = r2.exec_time_ns

    out = np.empty((B, V, N, D), np.float32)
    for c in range(NCORES):
        lo = c * NPC
        hi = min((c + 1) * NPC, N)
        if lo >= N:
            continue
        oc = r2.results[c]["outT"].reshape(P, B, V, NPC).astype(np.float32)
        out[:, :, lo:hi, :] = oc[:, :, :, :hi - lo].transpose(1, 2, 3, 0)
    return out


# revision 34
# speedup vs baseline: 1.0020x; 1.0020x over previous
"""Trainium2 Bass kernel for nn_GATv2Layer4View (GAT message passing + inter-view MHA).

Self-contained: kernel(**inputs) -> np.ndarray [2, 4, 10000, 128] float32.

Math (faithful to reference):
  scores[e,h] = mean_bv(s_src[bv, src[e], h] + s_dst[bv, dst[e], h])   (node-separable)
  w = softmax(scores, axis=0) = ea[src]*eb[dst]/Z;  Z computed HOST-side from
  ea/eb (free) and folded into the in_proj columns.

Launch 1 (node-sharded, 1280 nodes/core), 3 phases to avoid ACT table thrash:
  A: h (d-major) for all 8 bv per tile -> leaky (ACT Lrelu) -> hl_all
  B: bv-sum tree -> att-scaled -> head-select matmul -> Exp -> ea/eb node-major
  C: h (node-major, stationary-x matmuls) * ea -> hi/lo fp8 e4m3 rows (2048 B)
Launch 2+3 fused (dst-node-range sharded): per dst tile: contiguous self-loop
  block + dma_gather of edge rows; one-hot scatter matmuls (fp8 DoubleRow, the
  one-hots are HOST-precomputed inputs); *eb -> gatT via DMA-xbar transposes
  (SBUF only, no HBM round-trip); inter-view MHA: stationary-x qkv matmuls
  (node-major out), attention middle as b-batched broadcast outer products on
  DVE (all bf16 SBUF), out_proj via xbar transpose + matmul, ACT bias epilogue.
  Output d-major bf16; host reassembles layout + dtype (host work is free).
"""

import math
import numpy as np
import ml_dtypes

import concourse.bass as bass
import concourse.bacc as bacc
import concourse.mybir as mybir
import concourse.tile as tile
from concourse.bass_utils import run_bass_kernel_spmd
from concourse.masks import make_identity

P = 128
NCORES = 8
B, V, N, FIN = 2, 4, 10000, 64
H, F = 4, 32
D = H * F                      # 128
NEG_SLOPE = 0.2

NPC = 1280                     # nodes per core
TPC = NPC // P                 # 10 tiles per core
TBLN = NCORES * NPC            # 10240 table rows (>= N, covers pad tiles)
ROW = 2048                     # bytes per table row: 1024 hi fp8 + 1024 lo fp8

FP32 = mybir.dt.float32
BF16 = mybir.dt.bfloat16
FP8 = mybir.dt.float8e4
I16 = mybir.dt.int16

BF = ml_dtypes.bfloat16
E4 = ml_dtypes.float8_e4m3

RUN_KW = {}
EXEC_TIMES = {}


# --------------------------------------------------------------------------
# host-side edge preprocessing
# --------------------------------------------------------------------------
class EdgePlan:
    pass


def prep_edges(edge_index: np.ndarray) -> EdgePlan:
    ei = np.asarray(edge_index)
    src = ei[0].astype(np.int64)
    dst = ei[1].astype(np.int64)
    order = np.argsort(dst, kind="stable")
    ss, ds = src[order], dst[order]

    n_tiles = NCORES * TPC
    bounds = np.searchsorted(ds, np.minimum(np.arange(n_tiles + 1) * P, N))
    counts = np.diff(bounds)
    cmax = int(math.ceil(counts.max() / P))
    if (cmax + 1) % 2:         # C_T = cmax+1 chunks/tile incl identity: even
        cmax += 1
    C_T = cmax + 1

    idx_all = np.full((NCORES, TPC * cmax * P), N, np.int64)   # pad -> zero row
    rel_all = np.full((NCORES, TPC * C_T * P), 200.0, np.float32)
    for c in range(NCORES):
        for t in range(TPC):
            g = c * TPC + t
            k = bounds[g + 1] - bounds[g]
            idx_all[c, t * cmax * P:t * cmax * P + k] = ss[bounds[g]:bounds[g + 1]]
            o = (t * C_T + 1) * P
            rel_all[c, o:o + k] = ds[bounds[g]:bounds[g + 1]] - g * P
            rel_all[c, t * C_T * P:t * C_T * P + P] = np.arange(P)  # identity chunk
    plan = EdgePlan()
    plan.cmax = cmax
    plan.idx16 = [np.ascontiguousarray(idx_all[c].astype(np.int16)
                                       .reshape(-1, 16).T) for c in range(NCORES)]
    # host-built one-hot scatter matrices: S[p, ci*128 + j] = (rel[ci,p] == j)
    plan.sh = []
    n_chunks = TPC * C_T
    j = np.arange(P)
    for c in range(NCORES):
        R = rel_all[c].reshape(n_chunks, P)              # [chunk, partition]
        O = (R[:, :, None] == j[None, None, :]).astype(E4)
        plan.sh.append(np.ascontiguousarray(
            O.transpose(1, 0, 2).reshape(P, n_chunks * P)))
    return plan


# --------------------------------------------------------------------------
# launch 1: node-sharded table build (hi-lo fp8 rows + node-major ea/eb)
# --------------------------------------------------------------------------
def build_l1():
    nc = bacc.Bacc("TRN2", target_bir_lowering=False, debug=False,
                   num_devices=NCORES)
    xT = nc.dram_tensor("xT", [FIN, TPC * 8 * P], BF16, kind="ExternalInput")
    wT = nc.dram_tensor("wT", [FIN, D], BF16, kind="ExternalInput")
    attc = nc.dram_tensor("attc", [P, 2], FP32, kind="ExternalInput")
    ind4 = nc.dram_tensor("ind4", [P, 4], BF16, kind="ExternalInput")
    rows_out = nc.dram_tensor("rows", [NPC, ROW], FP8, kind="ExternalOutput")
    ee_out = nc.dram_tensor("ee", [NPC, 8], FP32, kind="ExternalOutput")

    with tile.TileContext(nc) as tc:
        with tc.tile_pool(name="one", bufs=1) as one, \
             tc.tile_pool(name="sb", bufs=4) as sb, \
             tc.tile_pool(name="pk", bufs=3) as pk, \
             tc.tile_pool(name="psA", bufs=2, space="PSUM") as psA, \
             tc.tile_pool(name="psB", bufs=2, space="PSUM") as psB, \
             tc.tile_pool(name="psS", bufs=1, space="PSUM") as psS:
            idf4 = one.tile([4, 4], FP32)
            make_identity(nc, idf4[:])
            xT_sb = one.tile([FIN, TPC * 8 * P], BF16)
            nc.sync.dma_start(xT_sb[:], xT.ap()[:])
            wT_sb = one.tile([FIN, D], BF16)
            nc.sync.dma_start(wT_sb[:], wT.ap()[:])
            att_sb = one.tile([P, 2], FP32)
            nc.sync.dma_start(att_sb[:], attc.ap()[:])
            ind_sb = one.tile([P, 4], BF16)
            nc.sync.dma_start(ind_sb[:], ind4.ap()[:])
            hl_all = one.tile([P, TPC * 8 * P], BF16)
            ee_all = one.tile([P, TPC, 8], FP32)

            # phase A: d-major h + leaky
            for t in range(TPC):
                c0 = t * 8 * P
                h_ps = psA.tile([P, 8 * P], FP32, tag="h")
                nc.tensor.matmul(h_ps[:, 0:512], wT_sb[:],
                                 xT_sb[:, c0:c0 + 512], start=True, stop=True)
                nc.tensor.matmul(h_ps[:, 512:1024], wT_sb[:],
                                 xT_sb[:, c0 + 512:c0 + 1024],
                                 start=True, stop=True)
                nc.scalar.activation(hl_all[:, c0:c0 + 8 * P], h_ps[:],
                                     mybir.ActivationFunctionType.Lrelu,
                                     alpha=NEG_SLOPE)

            # phase B: scores -> ea/eb node-major
            for t in range(TPC):
                c0 = t * 8 * P
                a1 = sb.tile([P, 512], BF16, tag="a1")
                nc.vector.tensor_add(a1[:], hl_all[:, c0:c0 + 512],
                                     hl_all[:, c0 + 512:c0 + 1024])
                a2 = sb.tile([P, 256], BF16, tag="a2")
                nc.vector.tensor_add(a2[:], a1[:, 0:256], a1[:, 256:512])
                a3 = sb.tile([P, P], FP32, tag="a3")
                nc.vector.tensor_add(a3[:], a2[:, 0:128], a2[:, 128:256])
                pp = sb.tile([P, 256], BF16, tag="pp")
                nc.vector.tensor_scalar_mul(pp[:, 0:128], a3[:], att_sb[:, 0:1])
                nc.vector.tensor_scalar_mul(pp[:, 128:256], a3[:], att_sb[:, 1:2])
                s2_ps = psS.tile([4, 256], FP32, tag="s2")
                nc.tensor.matmul(s2_ps[:], ind_sb[:], pp[:],
                                 start=True, stop=True)
                ee_row = sb.tile([4, 256], FP32, tag="eer")
                nc.scalar.activation(ee_row[:], s2_ps[:],
                                     mybir.ActivationFunctionType.Exp,
                                     scale=1.0 / 8.0)
                eT_ps = psS.tile([P, 4], FP32, tag="eT")
                nc.tensor.transpose(eT_ps[:], ee_row[:, 0:128], idf4[:])
                nc.vector.tensor_copy(ee_all[:, t, 0:4], eT_ps[:])
                eT2_ps = psS.tile([P, 4], FP32, tag="eT")
                nc.tensor.transpose(eT2_ps[:], ee_row[:, 128:256], idf4[:])
                nc.vector.tensor_copy(ee_all[:, t, 4:8], eT2_ps[:])
                nc.sync.dma_start(ee_out.ap()[t * P:(t + 1) * P, :],
                                  ee_all[:, t])

            # phase C: node-major h * ea -> hi/lo fp8 rows
            for t in range(TPC):
                c0 = t * 8 * P
                ea128 = sb.tile([P, P], BF16, tag="ea128")
                nc.vector.tensor_copy(
                    ea128[:].rearrange("p (h f) -> p h f", h=H),
                    ee_all[:, t, 0:4][:, :, None].to_broadcast([P, H, F]))
                pk32 = pk.tile([P, 8 * P], FP32, tag="pk32")
                for bv in range(8):
                    hn_ps = psB.tile([P, P], FP32, tag="hn")
                    nc.tensor.matmul(hn_ps[:],
                                     xT_sb[:, c0 + bv * P:c0 + (bv + 1) * P],
                                     wT_sb[:], start=True, stop=True)
                    nc.vector.tensor_tensor(
                        out=pk32[:, bv * P:(bv + 1) * P], in0=hn_ps[:],
                        in1=ea128[:], op=mybir.AluOpType.mult)
                packed = pk.tile([P, ROW], FP8, tag="packed")
                nc.scalar.copy(packed[:, 0:1024], pk32[:])
                nc.vector.tensor_tensor(out=packed[:, 1024:2048], in0=pk32[:],
                                        in1=packed[:, 0:1024],
                                        op=mybir.AluOpType.subtract)
                nc.sync.dma_start(rows_out.ap()[t * P:(t + 1) * P, :], packed[:])
    nc.compile()
    return nc


# --------------------------------------------------------------------------
# launch 2+3 fused: edge aggregation + inter-view MHA per dst-node range
# --------------------------------------------------------------------------
def build_l23(cmax: int, has_ipb: bool):
    C_T = cmax + 1
    n_chunks = TPC * C_T
    idx_cols = TPC * cmax * 8

    nc = bacc.Bacc("TRN2", target_bir_lowering=False, debug=False,
                   num_devices=NCORES, num_swdge_queues=2)
    tbl_in = nc.dram_tensor("table", [TBLN, ROW], FP8, kind="ExternalInput")
    tblk_in = nc.dram_tensor("tblk", [NPC, ROW], FP8, kind="ExternalInput")
    idx_in = nc.dram_tensor("idx16", [16, idx_cols], I16, kind="ExternalInput")
    sh_in = nc.dram_tensor("sh", [P, n_chunks * P], FP8, kind="ExternalInput")
    ee_in = nc.dram_tensor("ee", [NPC, 8], FP32, kind="ExternalInput")
    wiz_in = nc.dram_tensor("wiz", [P, 3 * D], BF16, kind="ExternalInput")
    wo_in = nc.dram_tensor("woT", [P, D], BF16, kind="ExternalInput")
    cb_in = nc.dram_tensor("cbias", [P, 1], FP32, kind="ExternalInput")
    ipb_in = nc.dram_tensor("ipb", [1, 3 * D], FP32, kind="ExternalInput")
    o_out = nc.dram_tensor("outT", [P, B * V * NPC], BF16, kind="ExternalOutput")

    with tile.TileContext(nc) as tc:
        with tc.tile_pool(name="one", bufs=1) as one, \
             tc.tile_pool(name="gp", bufs=2) as gp, \
             tc.tile_pool(name="sbf", bufs=2) as sbf, \
             tc.tile_pool(name="mh", bufs=3) as mh, \
             tc.tile_pool(name="accp", bufs=2, space="PSUM") as accp, \
             tc.tile_pool(name="psQ", bufs=2, space="PSUM") as psQ:
            idx_sb = one.tile([P, idx_cols], I16)
            for r in range(8):
                nc.sync.dma_start(idx_sb[16 * r:16 * (r + 1), :], idx_in.ap()[:])
            S_all = one.tile([P, n_chunks * P], FP8)
            nc.sync.dma_start(S_all[:], sh_in.ap()[:])
            ee_sb = one.tile([P, TPC, 8], FP32)
            nc.sync.dma_start(ee_sb[:],
                              ee_in.ap().rearrange("(t p) c -> p t c", p=P))
            wiz_sb = one.tile([P, 3 * D], BF16)
            nc.sync.dma_start(wiz_sb[:], wiz_in.ap()[:])
            wo_sb = one.tile([P, D], BF16)
            nc.sync.dma_start(wo_sb[:], wo_in.ap()[:])
            cb_sb = one.tile([P, 1], FP32)
            nc.sync.dma_start(cb_sb[:], cb_in.ap()[:])
            ones_sb = one.tile([P, 1], BF16)
            nc.vector.memset(ones_sb[:], 1.0)
            if has_ipb:
                ipb_row = one.tile([1, 3 * D], FP32)
                nc.sync.dma_start(ipb_row[:], ipb_in.ap()[:])
                ipb_rb = one.tile([1, 3 * D], BF16)
                nc.vector.tensor_copy(ipb_rb[:], ipb_row[:])
                ipb_sb = one.tile([P, 3 * D], BF16)
                nc.gpsimd.partition_broadcast(ipb_sb[:], ipb_rb[:])

            gatT_sb = one.tile([P, 8 * NPC], BF16)   # [d, (bv, n)]

            acc_t = {}

            def emit_scatter(t):
                # ---- gather + scatter-accumulate --------------------------
                g = gp.tile([P, C_T, ROW], FP8, tag="g")
                nc.sync.dma_start(g[:, 0, :], tblk_in.ap()[t * P:(t + 1) * P, :])
                h1 = cmax // 2
                nc.gpsimd.dma_gather(
                    out_ap=g[:, 1:1 + h1, :],
                    in_ap=tbl_in.ap()[:],
                    idxs_ap=idx_sb[:, t * cmax * 8:(t * cmax + h1) * 8],
                    num_idxs=h1 * P,
                    num_idxs_reg=h1 * P,
                    elem_size=ROW,
                    single_packet=False,
                    queue_num=0,
                )
                nc.gpsimd.dma_gather(
                    out_ap=g[:, 1 + h1:C_T, :],
                    in_ap=tbl_in.ap()[:],
                    idxs_ap=idx_sb[:, (t * cmax + h1) * 8:(t + 1) * cmax * 8],
                    num_idxs=(cmax - h1) * P,
                    num_idxs_reg=(cmax - h1) * P,
                    elem_size=ROW,
                    single_packet=False,
                    queue_num=1,
                )
                acc = accp.tile([P, 1024], FP32, tag="acc")
                npair = C_T // 2
                for j in range(npair):
                    base = (t * C_T + 2 * j) * P
                    S2 = S_all[:, base:base + 2 * P].rearrange(
                        "p (k e) -> p k e", k=2)
                    g2 = g[:, 2 * j:2 * j + 2, :]
                    first, last = (j == 0), (j == npair - 1)
                    for half in range(2):
                        nc.tensor.matmul(
                            acc[:, half * 512:(half + 1) * 512], S2,
                            g2[:, :, half * 512:(half + 1) * 512],
                            start=first, stop=False,
                            perf_mode=mybir.MatmulPerfMode.DoubleRow,
                            skip_group_check=True)
                    for half in range(2):
                        nc.tensor.matmul(
                            acc[:, half * 512:(half + 1) * 512], S2,
                            g2[:, :, 1024 + half * 512:1024 + (half + 1) * 512],
                            start=False, stop=last,
                            perf_mode=mybir.MatmulPerfMode.DoubleRow,
                            skip_group_check=True)
                acc_t[t] = acc

            stage_q = {}

            def emit_stage2(t):
                acc = acc_t.pop(t)
                # ---- finalize: * eb -> gatT via xbar transposes -----------
                eb128 = sbf.tile([P, P], BF16, tag="eb128")
                nc.vector.tensor_tensor(
                    out=eb128[:].rearrange("p (h f) -> p h f", h=H),
                    in0=ee_sb[:, t, 4:8][:, :, None].to_broadcast([P, H, F]),
                    in1=ones_sb[:][:, :, None].to_broadcast([P, H, F]),
                    op=mybir.AluOpType.mult)
                om = sbf.tile([P, 8, P], BF16, tag="om")
                nc.vector.tensor_tensor(
                    out=om[:], in0=acc[:].rearrange("p (v d) -> p v d", v=8),
                    in1=eb128[:, None, :].to_broadcast([P, 8, P]),
                    op=mybir.AluOpType.mult)
                nc.sync.dma_start_transpose(
                    gatT_sb[:].rearrange("p (v n) -> p v n", v=8)
                    [:, :, t * P:(t + 1) * P],
                    om[:].rearrange("p v d -> p (v d)"))
                # ---- inter-view MHA (both b batched) ----------------------
                # layouts chosen so (b h) collapses to one uniform-stride dim
                q_sb = mh.tile([P, V, B, P], BF16, tag="q")        # [a][b][hf]
                k_sb = mh.tile([P, B, H, V, F], BF16, tag="kk")    # [b][h][k][f]
                v_sb = mh.tile([P, B, H, F, V], BF16, tag="vv")    # [b][h][f][k]
                for b in range(B):
                    for vh in range(2):
                        qkv_ps = psQ.tile([P, 2, 512], FP32, tag="qkv")
                        for vi in range(2):
                            v = vh * 2 + vi
                            nc.tensor.matmul(
                                qkv_ps[:, vi, 0:384],
                                gatT_sb[:, (b * V + v) * NPC + t * P:
                                        (b * V + v) * NPC + (t + 1) * P],
                                wiz_sb[:], start=True, stop=True)
                        nc.vector.tensor_tensor(
                            out=q_sb[:, vh * 2:vh * 2 + 2, b, :],
                            in0=qkv_ps[:, :, 0:128],
                            in1=ones_sb[:][:, :, None].to_broadcast(
                                [P, 2, P]),
                            op=mybir.AluOpType.mult)
                        for vi in range(2):
                            v = vh * 2 + vi
                            nc.scalar.copy(
                                k_sb[:, b, :, v, :],
                                qkv_ps[:, vi, 128:256].rearrange(
                                    "p (h f) -> p h f", h=H))
                            nc.scalar.copy(
                                v_sb[:, b, :, :, v],
                                qkv_ps[:, vi, 256:384].rearrange(
                                    "p (h f) -> p h f", h=H))
                if has_ipb:
                    nc.vector.tensor_tensor(
                        out=q_sb[:].rearrange("p a b d -> p (a b) d"),
                        in0=q_sb[:].rearrange("p a b d -> p (a b) d"),
                        in1=ipb_sb[:, None, 0:128].to_broadcast(
                            [P, V * B, P]),
                        op=mybir.AluOpType.add)
                    nc.vector.tensor_tensor(
                        out=k_sb[:].rearrange("p b h k f -> p (b h) k f"),
                        in0=k_sb[:].rearrange("p b h k f -> p (b h) k f"),
                        in1=ipb_sb[:, 128:256].rearrange(
                            "p (h f) -> p h f", h=H).rearrange(
                            "p h f -> p h 1 f").to_broadcast([P, B * H, V, F]),
                        op=mybir.AluOpType.add)
                    nc.vector.tensor_tensor(
                        out=v_sb[:].rearrange("p b h f k -> p (b h) f k"),
                        in0=v_sb[:].rearrange("p b h f k -> p (b h) f k"),
                        in1=ipb_sb[:, 256:384].rearrange(
                            "p (h f) -> p h f", h=H).rearrange(
                            "p h f -> p h f 1").to_broadcast([P, B * H, F, V]),
                        op=mybir.AluOpType.add)
                stage_q[t] = (q_sb, k_sb, v_sb)

            def emit_stage3(t):
                q_sb, k_sb, v_sb = stage_q.pop(t)
                lg = mh.tile([P, V, B, H, V], FP32, tag="lg")  # [a][b][h][k]
                for a in range(V):
                    prod = mh.tile([P, B * H, V, F], BF16, tag="prod")
                    nc.vector.tensor_tensor(
                        out=prod[:],
                        in0=q_sb[:, a].rearrange(
                            "p b (h f) -> p (b h) f", h=H)[:, :, None, :]
                            .to_broadcast([P, B * H, V, F]),
                        in1=k_sb[:].rearrange("p b h k f -> p (b h) k f"),
                        op=mybir.AluOpType.mult)
                    nc.vector.tensor_reduce(
                        out=lg[:, a].rearrange("p b h k -> p (b h) k"),
                        in_=prod[:],
                        axis=mybir.AxisListType.X, op=mybir.AluOpType.add)
                ex = mh.tile([P, V * B * H, V], FP32, tag="ex")
                nc.scalar.activation(ex[:],
                                     lg[:].rearrange("p a b h k -> p (a b h) k"),
                                     mybir.ActivationFunctionType.Exp,
                                     scale=1.0 / math.sqrt(F))
                ssum = mh.tile([P, V * B * H], FP32, tag="ssum")
                nc.vector.tensor_reduce(
                    out=ssum[:], in_=ex[:],
                    axis=mybir.AxisListType.X, op=mybir.AluOpType.add)
                rcp = mh.tile([P, V * B * H], FP32, tag="rcp")
                nc.vector.reciprocal(rcp[:], ssum[:])
                at = mh.tile([P, V * B * H, V], BF16, tag="at")
                nc.vector.tensor_tensor(
                    out=at[:], in0=ex[:],
                    in1=rcp[:, :, None].to_broadcast([P, V * B * H, V]),
                    op=mybir.AluOpType.mult)
                obf = mh.tile([P, B, V, P], BF16, tag="obf")
                atv = at[:].rearrange("p (a bh) k -> p a bh k", a=V)
                for a in range(V):
                    pv = mh.tile([P, B * H, F, V], BF16, tag="pv")
                    nc.vector.tensor_tensor(
                        out=pv[:],
                        in0=atv[:, a][:, :, None, :].to_broadcast(
                            [P, B * H, F, V]),
                        in1=v_sb[:].rearrange("p b h f k -> p (b h) f k"),
                        op=mybir.AluOpType.mult)
                    with nc.allow_low_precision("4-term AV sum feeds bf16 out_proj"):
                        nc.vector.tensor_reduce(
                            out=obf[:, :, a, :],
                            in_=pv[:].rearrange("p bh f k -> p (bh f) k"),
                            axis=mybir.AxisListType.X, op=mybir.AluOpType.add)
                oT_sb = mh.tile([P, B, V, P], BF16, tag="oT")   # [d][b][a][n]
                nc.scalar.dma_start_transpose(
                    oT_sb[:].rearrange("p b a n -> p (b a) n"),
                    obf[:].rearrange("p b a d -> p (b a d)"))
                fin_ps = psQ.tile([P, 2, 512], FP32, tag="qkv")
                for b in range(B):
                    nc.tensor.matmul(fin_ps[:, b], wo_sb[:],
                                     oT_sb[:, b].rearrange("p a n -> p (a n)"),
                                     start=True, stop=True)
                outb = mh.tile([P, B, V, P], BF16, tag="outb")
                for b in range(B):
                    nc.scalar.activation(
                        outb[:, b].rearrange("p v n -> p (v n)"),
                        fin_ps[:, b],
                        mybir.ActivationFunctionType.Identity,
                        bias=cb_sb[:, 0:1])
                nc.scalar.dma_start(
                    o_out.ap().rearrange("p (b v n) -> p b v n",
                                         b=B, v=V)[:, :, :, t * P:(t + 1) * P],
                    outb[:])

            for t in range(TPC):
                emit_scatter(t)
                if t >= 1:
                    emit_stage2(t - 1)
                if t >= 2:
                    emit_stage3(t - 2)
            emit_stage2(TPC - 1)
            emit_stage3(TPC - 2)
            emit_stage3(TPC - 1)
    nc.compile()
    return nc


# --------------------------------------------------------------------------
# host orchestration
# --------------------------------------------------------------------------
_cache = {}


def _get(name, builder, *args):
    if name not in _cache:
        _cache[name] = builder(*args)
    return _cache[name]


def kernel(x, W, att, in_proj_w, in_proj_b, out_proj_w, out_proj_b, bias,
           edge_index):
    x = np.asarray(x, np.float32)
    W = np.asarray(W, np.float32)
    att = np.asarray(att, np.float32)
    in_proj_w = np.asarray(in_proj_w, np.float32)
    in_proj_b = np.asarray(in_proj_b, np.float32)
    out_proj_w = np.asarray(out_proj_w, np.float32)
    out_proj_b = np.asarray(out_proj_b, np.float32)
    bias = np.asarray(bias, np.float32)
    ei = np.asarray(edge_index)

    plan_key = ei.tobytes()
    if ("plan", plan_key) not in _cache:
        _cache[("plan", plan_key)] = prep_edges(ei)
    plan = _cache[("plan", plan_key)]

    # ---- launch 1 ----
    nc1 = _get("l1", build_l1)
    xf = x.reshape(NCORES, N, FIN)
    xpad = np.zeros((NCORES, TBLN, FIN), BF)
    xpad[:, :N, :] = xf.astype(BF)
    wT = np.ascontiguousarray(W.T.astype(BF))
    attc = np.zeros((P, 2), np.float32)
    attc[:, 0] = att[0, :, :F].reshape(-1)
    attc[:, 1] = att[0, :, F:].reshape(-1)
    ind4 = np.zeros((P, 4), BF)
    for h in range(H):
        ind4[h * F:(h + 1) * F, h] = 1.0
    in1 = []
    for c in range(NCORES):
        sl = xpad[:, c * NPC:(c + 1) * NPC, :]            # [8, NPC, 64]
        xT_c = np.ascontiguousarray(
            sl.reshape(8, TPC, P, FIN).transpose(3, 1, 0, 2).reshape(FIN, -1))
        in1.append({"xT": xT_c, "wT": wT, "attc": attc, "ind4": ind4})
    r1 = run_bass_kernel_spmd(nc1, in1, core_ids=list(range(NCORES)), **RUN_KW)
    EXEC_TIMES["launch1"] = r1.exec_time_ns

    # ---- host: Z + folded weights ----
    table = np.concatenate([r1.results[c]["rows"] for c in range(NCORES)])
    ee = np.concatenate([r1.results[c]["ee"] for c in range(NCORES)])  # [TBLN,8]
    ea = ee[:N, 0:4].astype(np.float64)
    eb = ee[:N, 4:8].astype(np.float64)
    src = np.concatenate([ei[0].astype(np.int64), np.arange(N)])
    dst = np.concatenate([ei[1].astype(np.int64), np.arange(N)])
    Z = (ea[src] * eb[dst]).sum(axis=0)                   # [H]
    rz = (1.0 / Z).astype(np.float32)
    rzvec = rz[np.arange(D) // F]                         # [128]
    wiz = np.ascontiguousarray((in_proj_w.T * rzvec[:, None]).astype(BF))
    woT = np.ascontiguousarray(out_proj_w.T.astype(BF))
    cbias = np.ascontiguousarray((out_proj_b + bias).reshape(P, 1))
    ipb = np.ascontiguousarray(in_proj_b.reshape(1, 3 * D))
    has_ipb = bool(np.any(in_proj_b))

    # ---- launch 2+3 fused ----
    nc2 = _get(("l23", plan.cmax, has_ipb), build_l23, plan.cmax, has_ipb)
    in2 = []
    for c in range(NCORES):
        in2.append({"table": table, "tblk": table[c * NPC:(c + 1) * NPC],
                    "idx16": plan.idx16[c], "sh": plan.sh[c],
                    "ee": ee[c * NPC:(c + 1) * NPC], "wiz": wiz, "woT": woT,
                    "cbias": cbias, "ipb": ipb})
    r2 = run_bass_kernel_spmd(nc2, in2, core_ids=list(range(NCORES)), **RUN_KW)
    EXEC_TIMES["launch23"] = r2.exec_time_ns

    out = np.empty((B, V, N, D), np.float32)
    for c in range(NCORES):
        lo = c * NPC
        hi = min((c + 1) * NPC, N)
        if lo >= N:
            continue
        oc = r2.results[c]["outT"].reshape(P, B, V, NPC).astype(np.float32)
        out[:, :, lo:hi, :] = oc[:, :, :, :hi - lo].transpose(1, 2, 3, 0)
    return out
